# revision 6
# baseline (speedup 1.0000x reference)
"""Trainium2 Bass kernel for nn_CrossAttention (efficient/linear attention over video frames).

Math per (b, f) frame (n = h*w = 4096 pixels, c=256 channels, hidden=512, 8 heads x 64):
    q   = Wq @ x_frame                     # [512, 4096]
    qs  = softmax over dim_head (64-channel groups of q)
    ctx = einsum over kv tokens (per batch, tiny)
    out = Wout @ (blockdiag(ctx)^T @ qs) * scale + bout
        = M' @ qs + bout     with   M'[o, c] = scale * sum_e ctx[h(o), d(o), e] * Wout[c, (h(o), e)]

Sharding: data-parallel over (b, f): 32 frames / 8 cores = 4 frames per core.
Each core redundantly computes the tiny kv path (k/v proj + k softmax + context + M')
for its batch on-device, then runs the per-frame pipeline:
  MM1 (f32r, full PE rate)  ->  ACT exp psum->sbuf bf16
  MMZ: block-diag-of-ones-replicated matmul = per-head softmax sums, pre-broadcast
       across each head's 64 partitions (sum + broadcast in one PE op)
  DVE reciprocal + bf16 multiply  ->  MM2 (bf16) -> ACT copy(+bias) -> DMA out.
"""

import os
import numpy as np

import concourse.bass as bass
import concourse.bacc as bacc
import concourse.mybir as mybir
import concourse.tile as tile
from concourse.bass_utils import run_bass_kernel_spmd
from concourse.masks import make_identity

F32 = mybir.dt.float32
F32R = mybir.dt.float32r
BF16 = mybir.dt.bfloat16
EXP = mybir.ActivationFunctionType.Exp
IDENT = mybir.ActivationFunctionType.Identity

HEADS, DH = 8, 64
C, HID = 256, 512          # channels, heads*dh
L, DC = 77, 768            # kv tokens, kv dim
B, F_TOT, N = 2, 16, 4096  # batches, frames, pixels/frame
NCORES = 8
FPC = F_TOT * B // NCORES  # frames per core = 4
NG = 4                     # column groups per frame (1024 cols each)
GW = N // NG               # group width = 1024
NT = GW // 512             # 512-col tiles per group = 2
SCALE = DH ** -0.5

LAST_RESULTS = None  # BassKernelResults of the most recent run (for test.py)


def _build(tc):
    nc = tc.nc
    xs = nc.dram_tensor("xs", [C, FPC, N], F32, kind="ExternalInput").ap()
    kvb = nc.dram_tensor("kvb", [L, DC], F32, kind="ExternalInput").ap()
    wq = nc.dram_tensor("wq", [HID, C], F32, kind="ExternalInput").ap()
    wkv = nc.dram_tensor("wkv", [2 * HID, DC], F32, kind="ExternalInput").ap()
    wout = nc.dram_tensor("wout", [C, HID], F32, kind="ExternalInput").ap()
    bo = nc.dram_tensor("bo", [C], F32, kind="ExternalInput").ap()
    out = nc.dram_tensor("out", [C, FPC, N], F32, kind="ExternalOutput").ap()

    singles = tc.alloc_tile_pool(name="singles", bufs=1)

    identity = singles.tile([128, 128], F32, name="identity", tag="identity")
    make_identity(nc, identity)

    # Block-diagonal ones, replicated: lhsT[k, m] = 1 iff k and m in same 64-block.
    # ones_rep^T @ E gives, at every output row m, the sum over the 64-row head
    # block containing m -> per-head softmax denominator already broadcast.
    ones_rep = singles.tile([128, 128], BF16, name="ones_rep", tag="ones_rep")
    nc.vector.memset(ones_rep, 0.0)
    nc.vector.memset(ones_rep[0:64, 0:64], 1.0)
    nc.vector.memset(ones_rep[64:128, 64:128], 1.0)

    bo_t = []
    for cc in range(2):
        t = singles.tile([128, 1], F32, name=f"bo{cc}", tag=f"bo{cc}")
        nc.sync.dma_start(out=t, in_=bo[cc * 128:(cc + 1) * 128].rearrange("(p o) -> p o", o=1))
        bo_t.append(t)

    # ---- weight transposes (PE transpose via identity) ----
    prep = tc.alloc_tile_pool(name="prep", bufs=1)
    pp = tc.alloc_tile_pool(name="prep_psum", bufs=2, space="PSUM")

    # WqT [c, o] as 2 c-chunk tiles [128, 512]
    wqt = [singles.tile([128, HID], BF16, name=f"wqt{kc}", tag=f"wqt{kc}")
           for kc in range(2)]
    for oc in range(4):
        wq_sb = prep.tile([128, C], F32, name=f"wq_sb{oc}", tag="wq_sb", bufs=2)
        nc.sync.dma_start(out=wq_sb, in_=wq[oc * 128:(oc + 1) * 128, :])
        for kc in range(2):
            ps = pp.tile([128, 128], F32, name="tps", tag="tps", bufs=2)
            nc.tensor.transpose(ps, wq_sb[:, kc * 128:(kc + 1) * 128], identity)
            nc.vector.tensor_copy(wqt[kc][:, oc * 128:(oc + 1) * 128], ps)

    # WkvT [c, o2] as 6 c-chunk tiles [128, 1024]
    wkvt = [prep.tile([128, 2 * HID], F32, name=f"wkvt{kc}", tag=f"wkvt{kc}")
            for kc in range(6)]
    for m in range(8):
        wkv_sb = prep.tile([128, DC], F32, name=f"wkv_sb{m}", tag="wkv_sb", bufs=2)
        nc.sync.dma_start(out=wkv_sb, in_=wkv[m * 128:(m + 1) * 128, :])
        for kc in range(6):
            ps = pp.tile([128, 128], F32, name="tps", tag="tps", bufs=2)
            nc.tensor.transpose(ps, wkv_sb[:, kc * 128:(kc + 1) * 128], identity)
            nc.vector.tensor_copy(wkvt[kc][:, m * 128:(m + 1) * 128], ps)

    # WoutT [o2, c] as 4 o2-chunk tiles [128, 256]
    woutt = [prep.tile([128, C], F32, name=f"woutt{oc}", tag=f"woutt{oc}")
             for oc in range(4)]
    for cc in range(2):
        wout_sb = prep.tile([128, HID], F32, name=f"wout_sb{cc}", tag="wout_sb", bufs=2)
        nc.sync.dma_start(out=wout_sb, in_=wout[cc * 128:(cc + 1) * 128, :])
        for oc in range(4):
            ps = pp.tile([128, 128], F32, name="tps", tag="tps", bufs=2)
            nc.tensor.transpose(ps, wout_sb[:, oc * 128:(oc + 1) * 128], identity)
            nc.vector.tensor_copy(woutt[oc][:, cc * 128:(cc + 1) * 128], ps)

    # kv tokens, transposed to [c, l]
    kv_sb = prep.tile([L, DC], F32, name="kv_sb", tag="kv_sb")
    nc.sync.dma_start(out=kv_sb, in_=kvb)
    kvt = [prep.tile([128, L], F32, name=f"kvt{kc}", tag=f"kvt{kc}") for kc in range(6)]
    for kc in range(6):
        ps = pp.tile([128, L], F32, name="tps", tag="tps", bufs=2)
        nc.tensor.transpose(ps, kv_sb[:, kc * 128:(kc + 1) * 128], identity[0:L, 0:L])
        nc.vector.tensor_copy(kvt[kc], ps)

    # ---- kv path: kvp = Wkv @ kv^T -> k softmax over tokens -> transposes ----
    ks = [prep.tile([128, L], F32, name=f"ks{j}", tag=f"ks{j}") for j in range(4)]
    vs = [prep.tile([128, L], F32, name=f"vs{j}", tag=f"vs{j}") for j in range(4)]
    for m in range(8):
        kvp_ps = pp.tile([128, L], F32, name="kvp_ps", tag="kvp_ps", bufs=2)
        for kc in range(6):
            nc.tensor.matmul(kvp_ps, wkvt[kc][:, m * 128:(m + 1) * 128], kvt[kc],
                             start=(kc == 0), stop=(kc == 5))
        if m < 4:  # k half: exp with per-row (token-axis) sums fused in
            kexp = prep.tile([128, L], F32, name="kexp", tag="kexp", bufs=2)
            zk = prep.tile([128, 1], F32, name="zk", tag="zk", bufs=2)
            nc.scalar.activation(kexp, kvp_ps, EXP, accum_out=zk)
            rk = prep.tile([128, 1], F32, name="rk", tag="rk", bufs=2)
            nc.vector.reciprocal(rk, zk)
            nc.vector.tensor_scalar_mul(ks[m], kexp, rk)
        else:  # v half: plain copy out of psum
            nc.scalar.copy(vs[m - 4], kvp_ps)

    kts = prep.tile([L, HID], F32, name="kts", tag="kts")
    vts = prep.tile([L, HID], F32, name="vts", tag="vts")
    for j in range(4):
        ps = pp.tile([L, 128], F32, name="tps", tag="tps", bufs=2)
        nc.tensor.transpose(ps, ks[j], identity)
        nc.vector.tensor_copy(kts[:, j * 128:(j + 1) * 128], ps)
        ps2 = pp.tile([L, 128], F32, name="tps", tag="tps", bufs=2)
        nc.tensor.transpose(ps2, vs[j], identity)
        nc.vector.tensor_copy(vts[:, j * 128:(j + 1) * 128], ps2)

    # ---- context^T (per 2-head chunk; off-diagonal blocks unused) and M' ----
    # mp[oc][o, c] = SCALE * sum_e ctxT[h(o)][e, d(o)] * WoutT[(h(o), e), c]
    mp = [singles.tile([128, C], BF16, name=f"mp{oc}", tag=f"mp{oc}") for oc in range(4)]
    for oc in range(4):
        ctx_ps = pp.tile([128, 128], F32, name="ctx_ps", tag="ctx_ps", bufs=1)
        nc.tensor.matmul(ctx_ps, vts[:, oc * 128:(oc + 1) * 128],
                         kts[:, oc * 128:(oc + 1) * 128], start=True, stop=True)
        blk = prep.tile([128, 128], F32, name="blk", tag="blk", bufs=2)
        nc.vector.memset(blk, 0.0)
        nc.vector.tensor_copy(blk[0:64, 0:64], ctx_ps[0:64, 0:64])
        nc.vector.tensor_copy(blk[64:128, 64:128], ctx_ps[64:128, 64:128])
        mp_ps = pp.tile([128, C], F32, name="mp_ps", tag="mp_ps", bufs=1)
        nc.tensor.matmul(mp_ps, blk, woutt[oc], start=True, stop=True)
        with nc.allow_low_precision("M' in bf16 feeds a bf16 matmul"):
            nc.vector.tensor_scalar_mul(mp[oc], mp_ps, SCALE)

    pp.release()
    prep.release()

    # ---- main per-frame pipeline ----
    qp = tc.alloc_tile_pool(name="qp", bufs=2, space="PSUM")
    zp = tc.alloc_tile_pool(name="zp", bufs=1, space="PSUM")
    op = tc.alloc_tile_pool(name="op", bufs=1, space="PSUM")
    sb = tc.alloc_tile_pool(name="sb", bufs=2)

    for f in range(FPC):
        for g in range(NG):
            xt = []
            for cc in range(2):
                t = sb.tile([128, GW], BF16, name="xt", tag=f"xt{cc}", bufs=3)
                nc.gpsimd.dma_start(
                    out=t, in_=xs[cc * 128:(cc + 1) * 128, f, g * GW:(g + 1) * GW])
                xt.append(t)

            en = []
            for oc in range(4):
                q_ps = qp.tile([128, NT, 512], F32, name="q_ps", tag="q_ps")
                for nt in range(NT):
                    for kc in range(2):
                        nc.tensor.matmul(
                            q_ps[:, nt, :],
                            wqt[kc][:, oc * 128:(oc + 1) * 128],
                            xt[kc][:, nt * 512:(nt + 1) * 512],
                            start=(kc == 0), stop=(kc == 1))
                e_t = sb.tile([128, NT, 512], BF16, name="e_t", tag="e_t", bufs=3)
                nc.scalar.activation(e_t, q_ps, EXP)
                z_ps = zp.tile([128, NT, 512], F32, name="z_ps", tag="z_ps")
                for nt in range(NT):
                    nc.tensor.matmul(z_ps[:, nt, :], ones_rep, e_t[:, nt, :],
                                     start=True, stop=True)
                r_t = sb.tile([128, NT, 512], BF16, name="r_t", tag="r_t", bufs=3)
                with nc.allow_low_precision("softmax denominators in bf16"):
                    nc.vector.reciprocal(r_t, z_ps)
                en_t = sb.tile([128, NT, 512], BF16, name="en_t", tag=f"en{oc}", bufs=2)
                nc.vector.tensor_mul(en_t, e_t, r_t)
                en.append(en_t)

            for cc in range(2):
                o_ps = op.tile([128, NT, 512], F32, name="o_ps", tag="o_ps")
                for nt in range(NT):
                    for oc in range(4):
                        nc.tensor.matmul(o_ps[:, nt, :],
                                         mp[oc][:, cc * 128:(cc + 1) * 128],
                                         en[oc][:, nt, :],
                                         start=(oc == 0), stop=(oc == 3))
                o_sb = sb.tile([128, NT, 512], F32, name="o_sb", tag="o_sb", bufs=3)
                nc.scalar.activation(o_sb, o_ps, IDENT, bias=bo_t[cc])
                nc.sync.dma_start(
                    out=out[cc * 128:(cc + 1) * 128, f, g * GW:(g + 1) * GW],
                    in_=o_sb.rearrange("p a b -> p (a b)"))

    sb.release()
    op.release()
    zp.release()
    qp.release()
    singles.release()


_CACHED_NC = None


def _get_nc():
    global _CACHED_NC
    if _CACHED_NC is None:
        nc = bacc.Bacc("TRN2", target_bir_lowering=False, debug=False,
                       num_devices=NCORES)
        with tile.TileContext(nc) as tc:
            _build(tc)
        nc.compile()
        _CACHED_NC = nc
    return _CACHED_NC


def kernel(x, kv, Wq, Wkv, Wout, bout):
    """Full-input entry point. x: (2,256,16,64,64) f32 -> (2,256,16,64,64) f32."""
    global LAST_RESULTS
    x = np.ascontiguousarray(np.asarray(x, dtype=np.float32))
    kv = np.ascontiguousarray(np.asarray(kv, dtype=np.float32))
    Wq = np.ascontiguousarray(np.asarray(Wq, dtype=np.float32))
    Wkv = np.ascontiguousarray(np.asarray(Wkv, dtype=np.float32))
    Wout = np.ascontiguousarray(np.asarray(Wout, dtype=np.float32))
    bout = np.ascontiguousarray(np.asarray(bout, dtype=np.float32))

    b, c, f_tot, hh, ww = x.shape
    assert (b, c, f_tot, hh * ww) == (B, C, F_TOT, N)
    xr = x.reshape(B, C, F_TOT, N)

    fpb = NCORES // B  # cores per batch
    in_maps = []
    for core in range(NCORES):
        bb = core // fpb
        f0 = (core % fpb) * FPC
        in_maps.append({
            "xs": np.ascontiguousarray(xr[bb, :, f0:f0 + FPC, :]),
            "kvb": kv[bb],
            "wq": Wq, "wkv": Wkv, "wout": Wout, "bo": bout,
        })

    nc = _get_nc()
    trace = bool(int(os.environ.get("KERNEL_TRACE", "0")))
    res = run_bass_kernel_spmd(nc, in_maps, core_ids=list(range(NCORES)),
                               trace=trace)
    LAST_RESULTS = res

    out = np.empty((B, C, F_TOT, N), dtype=np.float32)
    for core in range(NCORES):
        bb = core // fpb
        f0 = (core % fpb) * FPC
        out[bb, :, f0:f0 + FPC, :] = res.results[core]["out"]
    return out.reshape(B, C, F_TOT, hh, ww)


# revision 7
# speedup vs baseline: 1.2514x; 1.2514x over previous
"""Trainium2 Bass kernel for nn_CrossAttention (efficient/linear attention over video frames).

Math per (b, f) frame (n = h*w = 4096 pixels, c=256 channels, hidden=512, 8 heads x 64):
    q   = Wq @ x_frame                     # [512, 4096]
    qs  = softmax over dim_head (64-channel groups of q)
    ctx = einsum over kv tokens (per batch, tiny)
    out = Wout @ (blockdiag(ctx)^T @ qs) * scale + bout
        = M' @ qs + bout     with   M'[o, c] = scale * sum_e ctx[h(o), d(o), e] * Wout[c, (h(o), e)]

Sharding: data-parallel over (b, f): 32 frames / 8 cores = 4 frames per core.
Each core redundantly computes the tiny kv path (k/v proj + k softmax + context + M')
for its batch on-device, then runs the per-frame pipeline:
  MM1 (f32r, full PE rate)  ->  ACT exp psum->sbuf bf16
  MMZ: block-diag-of-ones-replicated matmul = per-head softmax sums, pre-broadcast
       across each head's 64 partitions (sum + broadcast in one PE op)
  DVE reciprocal + bf16 multiply  ->  MM2 (bf16) -> ACT copy(+bias) -> DMA out.
"""

import os
import numpy as np

import concourse.bass as bass
import concourse.bacc as bacc
import concourse.mybir as mybir
import concourse.tile as tile
from concourse.bass_utils import run_bass_kernel_spmd
from concourse.masks import make_identity

F32 = mybir.dt.float32
F32R = mybir.dt.float32r
BF16 = mybir.dt.bfloat16
EXP = mybir.ActivationFunctionType.Exp
LN = mybir.ActivationFunctionType.Ln
IDENT = mybir.ActivationFunctionType.Identity

HEADS, DH = 8, 64
C, HID = 256, 512          # channels, heads*dh
L, DC = 77, 768            # kv tokens, kv dim
B, F_TOT, N = 2, 16, 4096  # batches, frames, pixels/frame
NCORES = 8
FPC = F_TOT * B // NCORES  # frames per core = 4
NG = 4                     # column groups per frame (1024 cols each)
GW = N // NG               # group width = 1024
NT = GW // 512             # 512-col tiles per group = 2
SCALE = DH ** -0.5

LAST_RESULTS = None  # BassKernelResults of the most recent run (for test.py)


def _build(tc):
    nc = tc.nc
    xs = nc.dram_tensor("xs", [C, FPC, N], F32, kind="ExternalInput").ap()
    kvb = nc.dram_tensor("kvb", [L, DC], F32, kind="ExternalInput").ap()
    wq = nc.dram_tensor("wq", [HID, C], F32, kind="ExternalInput").ap()
    wkv = nc.dram_tensor("wkv", [2 * HID, DC], F32, kind="ExternalInput").ap()
    wout = nc.dram_tensor("wout", [C, HID], F32, kind="ExternalInput").ap()
    bo = nc.dram_tensor("bo", [C], F32, kind="ExternalInput").ap()
    out = nc.dram_tensor("out", [C, FPC, N], F32, kind="ExternalOutput").ap()

    singles = tc.alloc_tile_pool(name="singles", bufs=1)

    identity = singles.tile([128, 128], F32, name="identity", tag="identity")
    make_identity(nc, identity)

    # Block-diagonal ones, replicated: lhsT[k, m] = 1 iff k and m in same 64-block.
    # ones_rep^T @ E gives, at every output row m, the sum over the 64-row head
    # block containing m -> per-head softmax denominator already broadcast.
    ones_rep = singles.tile([128, 128], BF16, name="ones_rep", tag="ones_rep")
    nc.vector.memset(ones_rep, 0.0)
    nc.vector.memset(ones_rep[0:64, 0:64], 1.0)
    nc.vector.memset(ones_rep[64:128, 64:128], 1.0)

    bo_t = []
    for cc in range(2):
        t = singles.tile([128, 1], F32, name=f"bo{cc}", tag=f"bo{cc}")
        nc.sync.dma_start(out=t, in_=bo[cc * 128:(cc + 1) * 128].rearrange("(p o) -> p o", o=1))
        bo_t.append(t)

    # ---- weight transposes (PE transpose via identity) ----
    prep = tc.alloc_tile_pool(name="prep", bufs=1)
    pp = tc.alloc_tile_pool(name="prep_psum", bufs=2, space="PSUM")

    # WqT [c, o] as 2 c-chunk tiles [128, 512]
    wqt = [singles.tile([128, HID], BF16, name=f"wqt{kc}", tag=f"wqt{kc}")
           for kc in range(2)]
    for oc in range(4):
        wq_sb = prep.tile([128, C], F32, name=f"wq_sb{oc}", tag="wq_sb", bufs=2)
        nc.sync.dma_start(out=wq_sb, in_=wq[oc * 128:(oc + 1) * 128, :])
        for kc in range(2):
            ps = pp.tile([128, 128], F32, name="tps", tag="tps", bufs=2)
            nc.tensor.transpose(ps, wq_sb[:, kc * 128:(kc + 1) * 128], identity)
            nc.vector.tensor_copy(wqt[kc][:, oc * 128:(oc + 1) * 128], ps)

    # WkvT [c, o2] as 6 c-chunk tiles [128, 1024]
    wkvt = [prep.tile([128, 2 * HID], F32, name=f"wkvt{kc}", tag=f"wkvt{kc}")
            for kc in range(6)]
    for m in range(8):
        wkv_sb = prep.tile([128, DC], F32, name=f"wkv_sb{m}", tag="wkv_sb", bufs=2)
        nc.sync.dma_start(out=wkv_sb, in_=wkv[m * 128:(m + 1) * 128, :])
        for kc in range(6):
            ps = pp.tile([128, 128], F32, name="tps", tag="tps", bufs=2)
            nc.tensor.transpose(ps, wkv_sb[:, kc * 128:(kc + 1) * 128], identity)
            nc.vector.tensor_copy(wkvt[kc][:, m * 128:(m + 1) * 128], ps)

    # WoutT [o2, c] as 4 o2-chunk tiles [128, 256]
    woutt = [prep.tile([128, C], F32, name=f"woutt{oc}", tag=f"woutt{oc}")
             for oc in range(4)]
    for cc in range(2):
        wout_sb = prep.tile([128, HID], F32, name=f"wout_sb{cc}", tag="wout_sb", bufs=2)
        nc.sync.dma_start(out=wout_sb, in_=wout[cc * 128:(cc + 1) * 128, :])
        for oc in range(4):
            ps = pp.tile([128, 128], F32, name="tps", tag="tps", bufs=2)
            nc.tensor.transpose(ps, wout_sb[:, oc * 128:(oc + 1) * 128], identity)
            nc.vector.tensor_copy(woutt[oc][:, cc * 128:(cc + 1) * 128], ps)

    # kv tokens, transposed to [c, l]
    kv_sb = prep.tile([L, DC], F32, name="kv_sb", tag="kv_sb")
    nc.sync.dma_start(out=kv_sb, in_=kvb)
    kvt = [prep.tile([128, L], F32, name=f"kvt{kc}", tag=f"kvt{kc}") for kc in range(6)]
    for kc in range(6):
        ps = pp.tile([128, L], F32, name="tps", tag="tps", bufs=2)
        nc.tensor.transpose(ps, kv_sb[:, kc * 128:(kc + 1) * 128], identity[0:L, 0:L])
        nc.vector.tensor_copy(kvt[kc], ps)

    # ---- kv path: kvp = Wkv @ kv^T -> k softmax over tokens -> transposes ----
    ks = [prep.tile([128, L], F32, name=f"ks{j}", tag=f"ks{j}") for j in range(4)]
    vs = [prep.tile([128, L], F32, name=f"vs{j}", tag=f"vs{j}") for j in range(4)]
    for m in range(8):
        kvp_ps = pp.tile([128, L], F32, name="kvp_ps", tag="kvp_ps", bufs=2)
        for kc in range(6):
            nc.tensor.matmul(kvp_ps, wkvt[kc][:, m * 128:(m + 1) * 128], kvt[kc],
                             start=(kc == 0), stop=(kc == 5))
        if m < 4:  # k half: exp with per-row (token-axis) sums fused in
            kexp = prep.tile([128, L], F32, name="kexp", tag="kexp", bufs=2)
            zk = prep.tile([128, 1], F32, name="zk", tag="zk", bufs=2)
            nc.scalar.activation(kexp, kvp_ps, EXP, accum_out=zk)
            rk = prep.tile([128, 1], F32, name="rk", tag="rk", bufs=2)
            nc.vector.reciprocal(rk, zk)
            nc.vector.tensor_scalar_mul(ks[m], kexp, rk)
        else:  # v half: plain copy out of psum
            nc.scalar.copy(vs[m - 4], kvp_ps)

    kts = prep.tile([L, HID], F32, name="kts", tag="kts")
    vts = prep.tile([L, HID], F32, name="vts", tag="vts")
    for j in range(4):
        ps = pp.tile([L, 128], F32, name="tps", tag="tps", bufs=2)
        nc.tensor.transpose(ps, ks[j], identity)
        nc.vector.tensor_copy(kts[:, j * 128:(j + 1) * 128], ps)
        ps2 = pp.tile([L, 128], F32, name="tps", tag="tps", bufs=2)
        nc.tensor.transpose(ps2, vs[j], identity)
        nc.vector.tensor_copy(vts[:, j * 128:(j + 1) * 128], ps2)

    # ---- context^T (per 2-head chunk; off-diagonal blocks unused) and M' ----
    # mp[oc][o, c] = SCALE * sum_e ctxT[h(o)][e, d(o)] * WoutT[(h(o), e), c]
    mp = [singles.tile([128, C], BF16, name=f"mp{oc}", tag=f"mp{oc}") for oc in range(4)]
    for oc in range(4):
        ctx_ps = pp.tile([128, 128], F32, name="ctx_ps", tag="ctx_ps", bufs=1)
        nc.tensor.matmul(ctx_ps, vts[:, oc * 128:(oc + 1) * 128],
                         kts[:, oc * 128:(oc + 1) * 128], start=True, stop=True)
        blk = prep.tile([128, 128], F32, name="blk", tag="blk", bufs=2)
        nc.vector.memset(blk, 0.0)
        nc.vector.tensor_copy(blk[0:64, 0:64], ctx_ps[0:64, 0:64])
        nc.vector.tensor_copy(blk[64:128, 64:128], ctx_ps[64:128, 64:128])
        mp_ps = pp.tile([128, C], F32, name="mp_ps", tag="mp_ps", bufs=1)
        nc.tensor.matmul(mp_ps, blk, woutt[oc], start=True, stop=True)
        with nc.allow_low_precision("M' in bf16 feeds a bf16 matmul"):
            nc.vector.tensor_scalar_mul(mp[oc], mp_ps, SCALE)

    pp.release()
    prep.release()

    # ---- main per-frame pipeline ----
    qp = tc.alloc_tile_pool(name="qp", bufs=2, space="PSUM")
    zp = tc.alloc_tile_pool(name="zp", bufs=1, space="PSUM")
    op = tc.alloc_tile_pool(name="op", bufs=1, space="PSUM")
    sb = tc.alloc_tile_pool(name="sb", bufs=2)

    for f in range(FPC):
        for g in range(NG):
            xt = []
            for cc in range(2):
                t = sb.tile([128, GW], BF16, name="xt", tag=f"xt{cc}", bufs=3)
                nc.gpsimd.dma_start(
                    out=t, in_=xs[cc * 128:(cc + 1) * 128, f, g * GW:(g + 1) * GW])
                xt.append(t)

            en = []
            for oc in range(4):
                q_ps = qp.tile([128, NT, 512], F32, name="q_ps", tag="q_ps")
                for nt in range(NT):
                    for kc in range(2):
                        nc.tensor.matmul(
                            q_ps[:, nt, :],
                            wqt[kc][:, oc * 128:(oc + 1) * 128],
                            xt[kc][:, nt * 512:(nt + 1) * 512],
                            start=(kc == 0), stop=(kc == 1))
                e_t = sb.tile([128, NT, 512], BF16, name="e_t", tag="e_t", bufs=3)
                nc.scalar.activation(e_t, q_ps, EXP)
                z_ps = zp.tile([128, NT, 512], F32, name="z_ps", tag="z_ps")
                for nt in range(NT):
                    nc.tensor.matmul(z_ps[:, nt, :], ones_rep, e_t[:, nt, :],
                                     start=True, stop=True)
                lz_t = sb.tile([128, NT, 512], BF16, name="lz_t", tag="lz_t", bufs=3)
                nc.scalar.activation(lz_t, z_ps, LN)
                r_t = sb.tile([128, NT, 512], BF16, name="r_t", tag="r_t", bufs=3)
                nc.scalar.activation(r_t, lz_t, EXP, scale=-1.0)
                en_t = sb.tile([128, NT, 512], BF16, name="en_t", tag=f"en{oc}", bufs=2)
                nc.vector.tensor_mul(en_t, e_t, r_t)
                en.append(en_t)

            for cc in range(2):
                o_ps = op.tile([128, NT, 512], F32, name="o_ps", tag="o_ps")
                for nt in range(NT):
                    for oc in range(4):
                        nc.tensor.matmul(o_ps[:, nt, :],
                                         mp[oc][:, cc * 128:(cc + 1) * 128],
                                         en[oc][:, nt, :],
                                         start=(oc == 0), stop=(oc == 3))
                o_sb = sb.tile([128, NT, 512], F32, name="o_sb", tag="o_sb", bufs=3)
                nc.vector.tensor_scalar_add(o_sb, o_ps, bo_t[cc])
                nc.sync.dma_start(
                    out=out[cc * 128:(cc + 1) * 128, f, g * GW:(g + 1) * GW],
                    in_=o_sb.rearrange("p a b -> p (a b)"))

    sb.release()
    op.release()
    zp.release()
    qp.release()
    singles.release()


_CACHED_NC = None


def _get_nc():
    global _CACHED_NC
    if _CACHED_NC is None:
        nc = bacc.Bacc("TRN2", target_bir_lowering=False, debug=False,
                       num_devices=NCORES)
        with tile.TileContext(nc) as tc:
            _build(tc)
        nc.compile()
        _CACHED_NC = nc
    return _CACHED_NC


def kernel(x, kv, Wq, Wkv, Wout, bout):
    """Full-input entry point. x: (2,256,16,64,64) f32 -> (2,256,16,64,64) f32."""
    global LAST_RESULTS
    x = np.ascontiguousarray(np.asarray(x, dtype=np.float32))
    kv = np.ascontiguousarray(np.asarray(kv, dtype=np.float32))
    Wq = np.ascontiguousarray(np.asarray(Wq, dtype=np.float32))
    Wkv = np.ascontiguousarray(np.asarray(Wkv, dtype=np.float32))
    Wout = np.ascontiguousarray(np.asarray(Wout, dtype=np.float32))
    bout = np.ascontiguousarray(np.asarray(bout, dtype=np.float32))

    b, c, f_tot, hh, ww = x.shape
    assert (b, c, f_tot, hh * ww) == (B, C, F_TOT, N)
    xr = x.reshape(B, C, F_TOT, N)

    fpb = NCORES // B  # cores per batch
    in_maps = []
    for core in range(NCORES):
        bb = core // fpb
        f0 = (core % fpb) * FPC
        in_maps.append({
            "xs": np.ascontiguousarray(xr[bb, :, f0:f0 + FPC, :]),
            "kvb": kv[bb],
            "wq": Wq, "wkv": Wkv, "wout": Wout, "bo": bout,
        })

    nc = _get_nc()
    trace = bool(int(os.environ.get("KERNEL_TRACE", "0")))
    res = run_bass_kernel_spmd(nc, in_maps, core_ids=list(range(NCORES)),
                               trace=trace)
    LAST_RESULTS = res

    out = np.empty((B, C, F_TOT, N), dtype=np.float32)
    for core in range(NCORES):
        bb = core // fpb
        f0 = (core % fpb) * FPC
        out[bb, :, f0:f0 + FPC, :] = res.results[core]["out"]
    return out.reshape(B, C, F_TOT, hh, ww)
